# revision 9
# baseline (speedup 1.0000x reference)
"""LocalAttention Trainium2 Bass kernel.

Problem: x (2, 2048, 1024) f32 -> QKV proj (16 heads, d=64), local-window
attention (|i-j| <= 128), output projection.

Sharding (no collectives): 8 shards = 2 batches x 4 seq-chunks of 512 q rows.
Each core gets a uniform 768-row KV halo slice [qs-128, qs+640), zero-padded
at the sequence edges, so the q rows sit at fixed offset 128 inside the kv
slice on every core (SPMD-uniform band positions; the mask also kills the
padded rows). Attention runs banded: per head and per 256-wide q slice, only
the 4 kv tiles (512 rows) covering the window are computed. Host
pre-transposes the x slices (x^T layout is required for PE contraction over
the embedding dim); output shards are disjoint row slices concatenated on
the host.

Device layouts per core:
  xtq [1024, 512]  x^T for q rows      xtkv [1024, 768]  x^T for kv rows
  QT  [1024, 512]  (head*64+d) x q     KT   [1024, 768]
  V~  [768, 16*65] per head 64 v-cols + ones col (softmax-sum trick)
  SET = exp(energy/32) * mask in [kv, q] layout -> PV matmul contracts kv on
  partitions; row 64 of the PV psum accumulates the softmax denominator.
  OT [1024, 512] normalized head outputs -> final projection vs wo.

All matmuls run as float32r (full-rate PE mode, fp32 storage); toggle with
KERNEL_F32R=0 for exact fp32 (4x slower PE).
"""

import os
from contextlib import ExitStack

import numpy as np

import concourse.bacc as bacc
import concourse.mybir as mybir
import concourse.tile as tile
from concourse.bass_utils import run_bass_kernel_spmd

F32 = mybir.dt.float32
F32R = mybir.dt.float32r
AF = mybir.ActivationFunctionType

EMB = 1024
NHEAD = 16
DHEAD = 64
WIN = 128
BATCH = 2
SEQ = 2048
NQ = 512          # q rows per core
NKV = 768         # kv rows per core (q rows + clamped 128 halo each side)
NT_E = EMB // 128   # 8 e-tiles
NT_KV = NKV // 128  # 6 kv-tiles
NT_Q = NQ // 128    # 4 q-tiles
QOFF = 128          # q row i == kv row i + QOFF, uniformly
NSL = 2             # q slices per core (256 wide)
QSL = NQ // NSL     # 256
SCALE = 1.0 / np.sqrt(EMB)       # energy / sqrt(emb)

USE_F32R = os.environ.get("KERNEL_F32R", "1") == "1"

LAST_RESULT = None  # BassKernelResults of the most recent run (for profiling)


def _body(ctx, tc, aps):
    nc = tc.nc
    MMDT = F32R if USE_F32R else F32
    r = lambda ap: ap  # matmul operands are typed F32R at declaration  # noqa: E731

    pw = ctx.enter_context(tc.tile_pool(name="w", bufs=9))
    pxt = ctx.enter_context(tc.tile_pool(name="xtot", bufs=8))
    pqt = ctx.enter_context(tc.tile_pool(name="qt", bufs=8))
    pkt = ctx.enter_context(tc.tile_pool(name="kt", bufs=8))
    pv = ctx.enter_context(tc.tile_pool(name="v", bufs=6))
    pset = ctx.enter_context(tc.tile_pool(name="set", bufs=14))
    pmask = ctx.enter_context(tc.tile_pool(name="mask", bufs=6))
    psml = ctx.enter_context(tc.tile_pool(name="small", bufs=1))
    prcb = ctx.enter_context(tc.tile_pool(name="rcb", bufs=3))
    pfin = ctx.enter_context(tc.tile_pool(name="fin", bufs=2))
    pbo = ctx.enter_context(tc.tile_pool(name="bo", bufs=1))
    pps = ctx.enter_context(tc.tile_pool(name="ps", bufs=5, space="PSUM"))
    ppv = ctx.enter_context(tc.tile_pool(name="pspv", bufs=3, space="PSUM"))

    ts = lambda i, s: slice(i * s, (i + 1) * s)  # noqa: E731

    # ---- persistent loads ----
    xtkv = []
    for t in range(NT_E):
        tk = pxt.tile([128, NKV], MMDT, tag="xtkv", bufs=8)
        nc.sync.dma_start(tk[:], aps["xtkv"][ts(t, 128), :])
        xtkv.append(tk)
    maskt = []
    for t in range(NT_KV):
        m = pmask.tile([128, NQ], F32, tag="mask")
        nc.sync.dma_start(m[:], aps["mask"][ts(t, 128), :])
        maskt.append(m)
    bqs = psml.tile([128, NT_E], F32, tag="bias", bufs=3)
    nc.sync.dma_start(bqs[:], aps["bqc"][:])
    bks = psml.tile([128, NT_E], F32, tag="bias", bufs=3)
    nc.sync.dma_start(bks[:], aps["bkc"][:])
    bvs = psml.tile([128, NT_E], F32, tag="bias", bufs=3)
    nc.sync.dma_start(bvs[:], aps["bvc"][:])
    bob = pbo.tile([128, EMB], F32, tag="bo")
    nc.sync.dma_start(bob[:], aps["bob"][:])

    def load_w(name):
        tiles = []
        for t in range(NT_E):
            w = pw.tile([128, EMB], MMDT, tag="w")
            nc.sync.dma_start(w[:], aps[name][ts(t, 128), :])
            tiles.append(w)
        return tiles

    # ---- QT = (x_q @ wq + bq)^T ----
    wq_t = load_w("wq")
    qt = []
    for t in range(NT_E):
        ps = pps.tile([128, NQ], F32, tag="ps")
        for k in range(NT_E):
            nc.tensor.matmul(ps[:], r(wq_t[k][:, ts(t, 128)]),
                 r(xtkv[k][:, QOFF:QOFF + NQ]),
                             start=(k == 0), stop=(k == NT_E - 1))
        q = pqt.tile([128, NQ], MMDT, tag="qt")
        nc.scalar.activation(q[:], ps[:], AF.Identity, bias=bqs[:, t:t + 1])
        qt.append(q)

    # ---- KT = (x_kv @ wk + bk)^T ----
    wk_t = load_w("wk")
    kt = []
    for t in range(NT_E):
        k_tile = pkt.tile([128, NKV], MMDT, tag="kt")
        for half in range(2):
            ps = pps.tile([128, NKV // 2], F32, tag="ps")
            for k in range(NT_E):
                nc.tensor.matmul(ps[:], r(wk_t[k][:, ts(t, 128)]),
                                 r(xtkv[k][:, ts(half, NKV // 2)]),
                                 start=(k == 0), stop=(k == NT_E - 1))
            nc.scalar.activation(k_tile[:, ts(half, NKV // 2)], ps[:],
                                 AF.Identity, bias=bks[:, t:t + 1])
        kt.append(k_tile)

    # ---- V~ = x_kv @ wv, strided per head with a ones column ----
    wv_t = load_w("wv")
    v = []
    for kvt in range(NT_KV):
        vt = pv.tile([128, NHEAD * (DHEAD + 1)], MMDT, tag="v")
        vt_r = vt[:].rearrange("p (h d) -> p h d", d=DHEAD + 1)
        nc.sync.dma_start(vt_r[:, :, DHEAD:DHEAD + 1],
                          aps["onesc"][:].unsqueeze(2))
        for half in range(2):
            ps = pps.tile([128, 512], F32, tag="ps")
            for k in range(NT_E):
                nc.tensor.matmul(ps[:], r(xtkv[k][:, ts(kvt, 128)]),
                                 r(wv_t[k][:, ts(half, 512)]),
                                 start=(k == 0), stop=(k == NT_E - 1))
            nc.vector.tensor_copy(
                vt_r[:, ts(half, 8), 0:DHEAD],
                ps[:].rearrange("p (h d) -> p h d", d=DHEAD))
        v.append(vt)

    # ---- attention per head ----
    wo_t = load_w("wo")  # prefetch for the final projection
    ot = [pxt.tile([128, NQ], MMDT, tag="ot", bufs=8, name=f"ot{i}")
          for i in range(NT_E)]
    # q slice s covers q [s*256, s*256+256) = kv rows [s*256+128, s*256+384);
    # its window reaches kv [s*256, s*256+512) = kv tiles [2s, 2s+4) exactly.
    for h in range(NHEAD):
        th, off = h // 2, (h % 2) * DHEAD
        for s in range(NSL):
            qsl = ts(s, QSL)
            sets = []
            for j in range(4):
                kvt = 2 * s + j
                pe = pps.tile([128, QSL], F32, tag="ps")
                nc.tensor.matmul(pe[:],
                                 r(kt[th][off:off + DHEAD, ts(kvt, 128)]),
                                 r(qt[th][off:off + DHEAD, qsl]),
                                 start=True, stop=True)
                st = pset.tile([128, QSL], MMDT, tag="set")
                nc.scalar.activation(st[:], pe[:], AF.Exp, scale=float(SCALE))
                nc.vector.tensor_mul(st[:], st[:], maskt[kvt][:, qsl])
                sets.append(st)
            po = ppv.tile([DHEAD + 1, QSL], F32, tag="pv")
            for j in range(4):
                nc.tensor.matmul(po[:],
                                 r(v[2 * s + j][:, h * 65:h * 65 + 65]),
                                 r(sets[j][:]),
                                 start=(j == 0), stop=(j == 3))
            rc = psml.tile([1, QSL], F32, tag="recip", bufs=3,
                           name=f"rc{h}_{s}")
            nc.vector.reciprocal(rc[:], po[DHEAD:DHEAD + 1, :])
            rb = prcb.tile([DHEAD, QSL], F32, tag="rcb", name=f"rb{h}_{s}")
            nc.sync.dma_start(
                rb[:], rc[:].unsqueeze(1).broadcast_to((1, DHEAD, QSL)))
            nc.vector.tensor_mul(ot[th][off:off + DHEAD, qsl],
                                 po[0:DHEAD, :], rb[:])
            nc.vector.tensor_scalar_add(ot[th][off:off + DHEAD, qsl],
                                        ot[th][off:off + DHEAD, qsl],
                                        bvs[off:off + DHEAD, th:th + 1])

    # ---- final projection: out = O @ wo + bo ----
    for q_i in range(NT_Q):
        fin = pfin.tile([128, EMB], F32, tag="fin")
        for ch in range(2):
            pf = pps.tile([128, 512], F32, tag="ps")
            for k in range(NT_E):
                nc.tensor.matmul(pf[:], r(ot[k][:, ts(q_i, 128)]),
                                 r(wo_t[k][:, ts(ch, 512)]),
                                 start=(k == 0), stop=(k == NT_E - 1))
            nc.vector.tensor_add(fin[:, ts(ch, 512)], pf[:],
                                 bob[:, ts(ch, 512)])
        nc.sync.dma_start(aps["out"][ts(q_i, 128), :], fin[:])


_NC_CACHE = {}


def _build_nc():
    key = ("nc", USE_F32R)
    if key in _NC_CACHE:
        return _NC_CACHE[key]
    nc = bacc.Bacc("TRN2", target_bir_lowering=False, debug=False,
                   enable_asserts=False, num_devices=8)
    MMDT = F32R if USE_F32R else F32
    aps = {}
    for name, shape, dt_ in [("xtkv", [EMB, NKV], MMDT),
                             ("mask", [NKV, NQ], F32),
                             ("wq", [EMB, EMB], MMDT), ("wk", [EMB, EMB], MMDT),
                             ("wv", [EMB, EMB], MMDT), ("wo", [EMB, EMB], MMDT),
                             ("bqc", [128, NT_E], F32), ("bkc", [128, NT_E], F32),
                             ("bvc", [128, NT_E], F32), ("bob", [128, EMB], F32),
                             ("onesc", [128, NHEAD], MMDT)]:
        aps[name] = nc.dram_tensor(name, shape, dt_, kind="ExternalInput").ap()
    aps["out"] = nc.dram_tensor("out", [NQ, EMB], F32,
                                kind="ExternalOutput").ap()
    with tile.TileContext(nc) as tc:
        with ExitStack() as ctx:
            _body(ctx, tc, aps)
    nc.compile()
    _NC_CACHE[key] = nc
    return nc


def _shard_inputs(x, wq, bq, wk, bk, wv, bv, wo, bo):
    x = np.asarray(x, dtype=np.float32)
    arrs = {n: np.ascontiguousarray(np.asarray(a, dtype=np.float32))
            for n, a in [("wq", wq), ("wk", wk), ("wv", wv), ("wo", wo)]}
    bq, bk, bv, bo = (np.asarray(b, dtype=np.float32) for b in (bq, bk, bv, bo))
    arrs["bqc"] = np.ascontiguousarray(bq.reshape(NT_E, 128).T)
    arrs["bkc"] = np.ascontiguousarray(bk.reshape(NT_E, 128).T)
    arrs["bvc"] = np.ascontiguousarray(bv.reshape(NT_E, 128).T)
    arrs["bob"] = np.ascontiguousarray(np.broadcast_to(bo, (128, EMB)))
    arrs["onesc"] = np.ones((128, NHEAD), dtype=np.float32)
    in_maps = []
    for core in range(8):
        b, c = core // 4, core % 4
        qs = c * NQ
        k0 = qs - QOFF  # first kv row; may be out of range (zero-padded)
        m = dict(arrs)
        xt = np.zeros((NKV, EMB), dtype=np.float32)
        lo, hi = max(0, k0), min(SEQ, k0 + NKV)
        xt[lo - k0:hi - k0, :] = x[b, lo:hi, :]
        m["xtkv"] = np.ascontiguousarray(xt.T)
        kpos = k0 + np.arange(NKV)[:, None]
        qpos = qs + np.arange(NQ)[None, :]
        m["mask"] = ((np.abs(kpos - qpos) <= WIN)
                     & (kpos >= 0) & (kpos < SEQ)).astype(np.float32)
        in_maps.append(m)
    return in_maps


def kernel(x, wq, bq, wk, bk, wv, bv, wo, bo):
    global LAST_RESULT
    nc = _build_nc()
    in_maps = _shard_inputs(x, wq, bq, wk, bk, wv, bv, wo, bo)
    res = run_bass_kernel_spmd(nc, in_maps, core_ids=list(range(8)))
    LAST_RESULT = res
    out = np.empty((BATCH, SEQ, EMB), dtype=np.float32)
    for core in range(8):
        b, c = core // 4, core % 4
        out[b, c * NQ:(c + 1) * NQ, :] = res.results[core]["out"]
    return out
